# revision 27
# baseline (speedup 1.0000x reference)
"""Multi-head attention (qkv proj + 2D RoPE + softmax attention + out proj)
for Trainium2, data-parallel over 8 NeuronCores (one batch element per core).

kernel(**inputs) takes FULL inputs (tensor (8,1024,1024), w_qkv (3072,1024),
w_proj (1024,1024)) and returns the FULL output (8,1024,1024).

v2 (626us -> ~450us): fp16 operands on the PE everywhere (same 1 cyc/row
as fp32r, half the DMA bytes), DMA issue spread across SP + ACT + Pool
sequencers with 4-way queue-split transfers, phase-1 half-group
pipelining, PV interleaved into the score/exp stream, and softmax
normalization restructured so the PE never waits on the slow DVE
reciprocal: the RAW denominator rows are broadcast by K=1 matmuls first,
and the reciprocal runs afterwards on the broadcast copy, off the PE
path.

Per-core dataflow (one batch element):
  phase 1: qT/kT = Wq^T.T @ xT (feature-on-partition, head-major rows),
           RoPE via cos/sin-permuted fp16 tables (swap DMAs issued by the
           Pool SWDGE so the SP queue never blocks on compute); v
           token-major fp16, scattered into a padded "vbuf" whose ones
           columns (built by Pool memsets) fuse the softmax denominator
           into the attention matmul. Half-groups of 4 pairs overlap RoPE
           with the next group's matmuls.
  phase 2: per head pair t: score matmuls (fp16, K=64) -> exp on ScalarE
           (fp16 p tiles, scale=1/8) with the PV i-steps interleaved two
           steps behind; at pair end the denominator rows are copied to
           SBUF (fast), broadcast across partitions by K=1 f32r matmuls,
           reciprocal'd on DVE off-path, and fp16 normalize-muls write oT
           from the immediately-evacuated raw o.
  phase 3: out = oT.T @ wpT with early-prefetched fp16 weights and
           ACT-issued split output DMAs.

HW-fault lore (CoreSim passes, device dies, error redacted): deferring
the s_ps broadcast/normalize across the next pair's scores crashes the
device, as do bufs=1 PSUM pools for PE-written/DVE-read tiles, fp16 K=1
matmuls (s3d3_mm_num_elements), and >512-wide matmul outputs (one PSUM
bank). Stay inside the patterns above.
"""
import numpy as np

import concourse.bass as bass
import concourse.bacc as bacc
import concourse.mybir as mybir
import concourse.tile as tile
from concourse import bass_utils

F32 = mybir.dt.float32
F32R = mybir.dt.float32r
F16 = mybir.dt.float16
AF = mybir.ActivationFunctionType

B, N, C = 8, 1024, 1024
H, HD = 16, 64
HEIGHT = WIDTH = 32
FREQ = 10000.0
NT = N // 128      # 8 row tiles
CT = C // 128      # 8 contraction tiles
PAIRS = H // 2     # 8 head pairs
VSLOT = 193        # [v_e 64 | one | one | z62 | v_o 64]


# ---------------------------------------------------------------- host prep
def _thetas():
    half = HD // 2
    ifreq = (1.0 / (FREQ ** (np.arange(half, dtype=np.float32) / np.float32(half)))).astype(np.float32)
    fh = np.arange(HEIGHT, dtype=np.float32)[:, None] * ifreq[None, :]
    fw = np.arange(WIDTH, dtype=np.float32)[:, None] * ifreq[None, :]
    th = np.broadcast_to(fh[:, None, :], (HEIGHT, WIDTH, half))
    tw = np.broadcast_to(fw[None, :, :], (HEIGHT, WIDTH, half))
    return np.concatenate([th, tw], axis=-1).reshape(N, HD)


def _host_tables():
    theta = _thetas()
    cos = np.cos(theta)                        # (N, HD) f64->f32
    sin = np.sin(theta)
    cosT2 = np.empty((128, N), np.float16)     # row 64p+d = cos(theta[:, d])
    sinP = np.empty((128, N), np.float16)      # permuted+signed sin
    for p in range(2):
        for d in range(HD):
            cosT2[64 * p + d] = cos[:, d]
            if d < 32:
                sinP[64 * p + d] = sin[:, d + 32]
            else:
                sinP[64 * p + d] = -sin[:, d - 32]
    # consts: [0:64]=1 (even-bcast mask), [64:320]=0, [320:384]=1 (odd mask)
    consts = np.zeros((128, 384), np.float32)
    consts[:, 0:64] = 1.0
    consts[:, 320:384] = 1.0
    consts16 = consts.astype(np.float16)
    # vbuf constant pattern (f16): ones at cols 64 and 97 of each slot
    vconst = np.zeros((128, VSLOT * PAIRS), np.float16)
    for s in range(PAIRS):
        vconst[:, VSLOT * s + 64] = 1.0   # even rowsum -> psum row 64
        vconst[:, VSLOT * s + 97] = 1.0   # odd rowsum -> psum row 32
    return cosT2, sinP, consts, consts16, vconst


def _host_weights(w_qkv, w_proj):
    w3 = np.asarray(w_qkv, np.float32).reshape(H, 3 * HD, C)
    wqT = np.ascontiguousarray(w3[:, 0:HD].reshape(H * HD, C).T.astype(np.float16))
    wkT = np.ascontiguousarray(w3[:, HD:2 * HD].reshape(H * HD, C).T.astype(np.float16))
    wvT = np.ascontiguousarray(w3[:, 2 * HD:3 * HD].reshape(H * HD, C).T.astype(np.float16))
    wpT = np.ascontiguousarray(np.asarray(w_proj, np.float32).T.astype(np.float16))
    return wqT, wkT, wvT, wpT


def _split_dma(nc, dst, src, ways=4, eng=None):
    """Issue one logical [P, X] DMA as `ways` partition-chunks so it spreads
    over several HW queues (per-queue descriptor rate is the bottleneck).
    `eng` picks the issuing engine (default SP/sync)."""
    eng = eng if eng is not None else nc.sync
    p = dst.shape[0]
    step = p // ways
    for w in range(ways):
        sl = slice(step * w, step * (w + 1))
        eng.dma_start(dst[sl], src[sl])


# ---------------------------------------------------------------- bass build
def build_kernel(nc):
    xT_d = nc.dram_tensor("xT", [C, N], F16, kind="ExternalInput").ap()
    wqT_d = nc.dram_tensor("wqT", [C, C], F16, kind="ExternalInput").ap()
    wkT_d = nc.dram_tensor("wkT", [C, C], F16, kind="ExternalInput").ap()
    wvT_d = nc.dram_tensor("wvT", [C, C], F16, kind="ExternalInput").ap()
    wpT_d = nc.dram_tensor("wpT", [C, C], F16, kind="ExternalInput").ap()
    cos_d = nc.dram_tensor("cosT2", [128, N], F16, kind="ExternalInput").ap()
    sinp_d = nc.dram_tensor("sinP", [128, N], F16, kind="ExternalInput").ap()
    con_d = nc.dram_tensor("consts", [128, 384], F32, kind="ExternalInput").ap()
    con16_d = nc.dram_tensor("consts16", [128, 384], F16,
                             kind="ExternalInput").ap()
    vcon_d = nc.dram_tensor("vconst", [128, VSLOT * PAIRS], F16,
                            kind="ExternalInput").ap()
    out_d = nc.dram_tensor("out", [N, C], F32, kind="ExternalOutput").ap()

    with tile.TileContext(nc) as tc:
        _body(tc, xT_d, wqT_d, wkT_d, wvT_d, wpT_d, cos_d, sinp_d, con_d,
              con16_d, vcon_d, out_d)
    return nc


def _body(tc, xT_d, wqT_d, wkT_d, wvT_d, wpT_d, cos_d, sinp_d, con_d,
          con16_d, vcon_d, out_d):
    nc = tc.nc

    with tc.tile_pool(name="persist", bufs=1) as persist, \
         tc.tile_pool(name="cpool", bufs=1) as cpool:
        qR = [persist.tile([128, N], F16, tag=f"qR{t}", name=f"qR{t}")
              for t in range(PAIRS)]
        kR = [persist.tile([128, N], F16, tag=f"kR{t}", name=f"kR{t}")
              for t in range(PAIRS)]
        vbuf = [persist.tile([128, VSLOT * PAIRS], F16, tag=f"vb{tn}",
                             name=f"vb{tn}") for tn in range(NT)]
        oT = [persist.tile([128, N], F16, tag=f"oT{t}", name=f"oT{t}")
              for t in range(PAIRS)]
        csb = cpool.tile([128, 384], F32R, tag="csb")
        _split_dma(nc, csb[:], con_d[:].bitcast(F32R), ways=4)
        csb16 = cpool.tile([128, 384], F16, tag="csb16")
        _split_dma(nc, csb16[:], con16_d[:], ways=4)

        # PE warm-up: fill the initial input-DMA window with dummy matmuls
        # on the constants tile so the HAM un-throttles before phase 1.
        with tc.tile_pool(name="warm", bufs=1, space="PSUM") as wpsum:
            wt = wpsum.tile([128, 384], F32, tag="warm", name="warm")
            for _ in range(32):
                nc.tensor.matmul(wt[:], csb[:, 0:128], csb[:],
                                 start=True, stop=True)

        # -------------------------------------------- phase 1
        with tc.tile_pool(name="tables", bufs=1) as tables, \
             tc.tile_pool(name="xbuf", bufs=1) as xpool, \
             tc.tile_pool(name="wstream", bufs=24) as wpool, \
             tc.tile_pool(name="rope", bufs=8) as rpool, \
             tc.tile_pool(name="pj", bufs=8, space="PSUM") as ppj:

            xT = [xpool.tile([128, N], F16, tag=f"xT{kc}", name=f"xT{kc}")
                  for kc in range(CT)]
            wq = []
            for kc in range(CT):
                _split_dma(nc, xT[kc][:], xT_d[128 * kc:128 * (kc + 1), :],
                           ways=4, eng=nc.scalar)
                w = wpool.tile([128, C], F16, tag="w", name="w")
                _split_dma(nc, w[:], wqT_d[128 * kc:128 * (kc + 1), :],
                           ways=4)
                wq.append(w)
            cos_sb = tables.tile([128, N], F16, tag="cos")
            _split_dma(nc, cos_sb[:], cos_d[:], ways=4)
            sinp_sb = tables.tile([128, N], F16, tag="sinp")
            _split_dma(nc, sinp_sb[:], sinp_d[:], ways=4)

        # stream weights: one [128, C] fp16 tile per kc, 4-way split DMAs.
        # Shared tag ring of 16 slots: two projections' weights in flight.
            def load_w(w_d):
                tiles = []
                for kc in range(CT):
                    w = wpool.tile([128, C], F16, tag="w", name="w")
                    _split_dma(nc, w[:],
                               w_d[128 * kc:128 * (kc + 1), :], ways=4)
                    tiles.append(w)
                return tiles

            def rope_chain(ps, dest, t, j):
                sl = slice(512 * j, 512 * (j + 1))
                qraw = rpool.tile([128, 512], F16, tag="qraw", name="qraw")
                nc.scalar.copy(qraw[:], ps[:])
                u = rpool.tile([128, 512], F16, tag="u")
                up = rpool.tile([128, 512], F16, tag="up")
                nc.vector.tensor_mul(u[:], qraw[:], sinp_sb[:, sl])
                nc.vector.tensor_mul(dest[t][:, sl], qraw[:], cos_sb[:, sl])
                for blk in range(4):
                    s = 32 * ((blk // 2) * 2 + 1 - (blk % 2))
                    d = 32 * blk
                    nc.gpsimd.dma_start(up[d:d + 32, :], u[s:s + 32, :])
                nc.vector.tensor_add(dest[t][:, sl], dest[t][:, sl], up[:])

            def proj_rope(w_tiles, dest, j):
                # two half-groups of 4 pairs; RoPE of one half overlaps the
                # other half's matmuls.
                for half in range(2):
                    ts = range(4 * half, 4 * half + 4)
                    ps = {t: ppj.tile([128, 512], F32, tag="pj", name="pj")
                          for t in ts}
                    for kc in range(CT):
                        for t in ts:
                            nc.tensor.matmul(
                                ps[t][:],
                                w_tiles[kc][:, 128 * t:128 * (t + 1)],
                                xT[kc][:, 512 * j:512 * (j + 1)],
                                start=(kc == 0), stop=(kc == CT - 1))
                    for t in ts:
                        rope_chain(ps[t][:], dest, t, j)

            def vproj(w_tiles, jc):
                for half in range(2):
                    tns = range(4 * half, 4 * half + 4)
                    ps = {tn: ppj.tile([128, 512], F32, tag="pj", name="pj")
                          for tn in tns}
                    for kc in range(CT):
                        for tn in tns:
                            nc.tensor.matmul(
                                ps[tn][:],
                                xT[kc][:, 128 * tn:128 * (tn + 1)],
                                w_tiles[kc][:, 512 * jc:512 * (jc + 1)],
                                start=(kc == 0), stop=(kc == CT - 1))
                    for tn in tns:
                        vsrc = ps[tn][:].rearrange("p (h c) -> p h c",
                                                   h=8, c=64)
                        vb = vbuf[tn][:].rearrange("p (s c) -> p s c",
                                                   s=PAIRS, c=VSLOT)
                        pv = slice(4 * jc, 4 * jc + 4)
                        nc.vector.tensor_copy(vb[:, pv, 0:64], vsrc[:, 0::2])
                        nc.vector.tensor_copy(vb[:, pv, 129:193],
                                              vsrc[:, 1::2])

            # vbuf constant pattern built on the (idle) Pool engine:
            # zeros + the two ones columns per head slot.
            for tn in range(NT):
                nc.gpsimd.memset(vbuf[tn][:], 0.0)
                for s in range(PAIRS):
                    nc.gpsimd.memset(vbuf[tn][:, VSLOT * s + 64:VSLOT * s + 65], 1.0)
                    nc.gpsimd.memset(vbuf[tn][:, VSLOT * s + 97:VSLOT * s + 98], 1.0)
            proj_rope(wq, qR, 0)
            wk = load_w(wkT_d)
            proj_rope(wq, qR, 1)
            proj_rope(wk, kR, 0)
            wv = load_w(wvT_d)
            proj_rope(wk, kR, 1)
            vproj(wv, 0)
            vproj(wv, 1)

        # -------------------------------------------- phase 2 + 3
        with tc.tile_pool(name="wp3pool", bufs=8) as wpool3:
            # prefetch phase-3 weights during phase 2
            wp = []
            for ct in range(CT):
                w = wpool3.tile([128, C], F16, tag="wp", name="wp")
                _split_dma(nc, w[:], wpT_d[128 * ct:128 * (ct + 1), :],
                           ways=4)
                wp.append(w)
            _phase2(tc, qR, kR, vbuf, oT, csb)
            _phase3(tc, oT, wp, out_d)


def _phase2(tc, qR, kR, vbuf, oT, csb):
    nc = tc.nc
    # PSUM budget (8 banks): sc 2x2 + oe 2 + oo 2 = 8
    with tc.tile_pool(name="psc", bufs=2, space="PSUM") as psc, \
         tc.tile_pool(name="poe", bufs=2, space="PSUM") as poe, \
         tc.tile_pool(name="poo", bufs=2, space="PSUM") as poo, \
         tc.tile_pool(name="pp", bufs=10) as ppool, \
         tc.tile_pool(name="ns", bufs=6) as nspool:

            def pair_stage(t):
                """Scores+exp with PV i-steps interleaved (lag 2), then the
                denominator broadcast + off-path reciprocal normalization."""
                pts = []
                ot = []
                for j in range(2):
                    o_e = poe.tile([65, 512], F32, tag="oe", name="oe")
                    o_o = poo.tile([128, 512], F32, tag="oo", name="oo")
                    ot.append((o_e, o_o))

                def pv_step(i):
                    for j in range(2):
                        sl = slice(512 * j, 512 * (j + 1))
                        o_e, o_o = ot[j]
                        vb = vbuf[i][:]
                        nc.tensor.matmul(
                            o_e[:], vb[:, VSLOT * t:VSLOT * t + 65],
                            pts[i][0][:, sl],
                            start=(i == 0), stop=(i == NT - 1))
                        nc.tensor.matmul(
                            o_o[:], vb[:, VSLOT * t + 65:VSLOT * (t + 1)],
                            pts[i][1][:, sl],
                            start=(i == 0), stop=(i == NT - 1))

                for i in range(NT):
                    p_e = ppool.tile([128, N], F16, tag="pe", name="pe")
                    p_o = ppool.tile([128, N], F16, tag="po", name="po")
                    for par, p_sb in ((0, p_e), (1, p_o)):
                        pr = slice(64 * par, 64 * par + 64)
                        sc = psc.tile([128, N], F32, tag="sc", name="sc")
                        for j in range(2):
                            sl = slice(512 * j, 512 * (j + 1))
                            nc.tensor.matmul(
                                sc[:, sl], kR[t][pr, 128 * i:128 * (i + 1)],
                                qR[t][pr, sl], start=True, stop=True)
                        nc.scalar.activation(p_sb[:], sc[:], AF.Exp,
                                             scale=0.125)
                    pts.append((p_e, p_o))
                    if i >= 2:
                        pv_step(i - 2)
                pv_step(NT - 2)
                pv_step(NT - 1)

                dens = []
                for j in range(2):
                    o_e, o_o = ot[j]
                    denS = nspool.tile([128, 512], F32R, tag="denS",
                                       name="denS")
                    with nc.allow_low_precision(reason="raw denom bcast"):
                        nc.vector.tensor_copy(denS[64:65, :], o_e[64:65, :])
                        nc.vector.tensor_copy(denS[32:33, :], o_o[32:33, :])
                    dens.append(denS)
                denbs = []
                for j in range(2):
                    denS = dens[j]
                    s_ps = psc.tile([128, N], F32, tag="sc", name="sps")
                    nc.tensor.matmul(s_ps[:, 0:512], csb[64:65, 0:128],
                                     denS[64:65, :], start=True, stop=False,
                                     tile_position=(64, 0))
                    nc.tensor.matmul(s_ps[:, 0:512], csb[32:33, 256:384],
                                     denS[32:33, :], start=False, stop=True,
                                     tile_position=(32, 0))
                    den_b = nspool.tile([128, 512], F32, tag="denb",
                                        name="denb")
                    nc.vector.tensor_copy(den_b[:], s_ps[:, 0:512])
                    denbs.append(den_b)
                for j in range(2):
                    sl = slice(512 * j, 512 * (j + 1))
                    o_e, o_o = ot[j]
                    # evacuate raw o to fp16 SBUF so the psum banks free fast
                    oraw = nspool.tile([128, 512], F16, tag="oraw",
                                       name="oraw")
                    nc.vector.tensor_copy(oraw[0:64, :], o_e[0:64, :])
                    nc.vector.tensor_copy(oraw[64:128, :], o_o[64:128, :])
                    sinv = nspool.tile([128, 512], F32R, tag="sinv",
                                       name="sinv")
                    with nc.allow_low_precision(reason="f32r recip"):
                        nc.vector.reciprocal(sinv[:], denbs[j][:])
                    nc.vector.tensor_mul(oT[t][:, sl], oraw[:], sinv[:])

            for t in range(PAIRS):
                pair_stage(t)


def _phase3(tc, oT, wp, out_d):
    nc = tc.nc
    with tc.tile_pool(name="ob", bufs=4) as opool, \
         tc.tile_pool(name="po3", bufs=4, space="PSUM") as ppo:
        for tn in range(NT):
            for jc in range(2):
                sl = slice(512 * jc, 512 * (jc + 1))
                ps = ppo.tile([128, 512], F32, tag="po3", name="po3")
                for ct in range(CT):
                    nc.tensor.matmul(ps[:],
                                     oT[ct][:, 128 * tn:128 * (tn + 1)],
                                     wp[ct][:, sl], start=(ct == 0),
                                     stop=(ct == CT - 1))
                ob = opool.tile([128, 512], F32, tag="ob")
                nc.scalar.copy(ob[:], ps[:])
                _split_dma(nc, out_d[128 * tn:128 * (tn + 1), sl], ob[:],
                           ways=4)


# ---------------------------------------------------------------- entry
_CACHE = {}


def _get_nc():
    if "nc" not in _CACHE:
        nc = bacc.Bacc("TRN2", target_bir_lowering=False, debug=False,
                       num_devices=B)
        build_kernel(nc)
        nc.compile()
        _CACHE["nc"] = nc
    return _CACHE["nc"]


def make_in_maps(tensor, w_qkv, w_proj):
    tensor = np.asarray(tensor, np.float32)
    wqT, wkT, wvT, wpT = _host_weights(w_qkv, w_proj)
    cosT2, sinP, consts, consts16, vconst = _host_tables()
    shared = {"wqT": wqT, "wkT": wkT, "wvT": wvT, "wpT": wpT,
              "cosT2": cosT2, "sinP": sinP, "consts": consts,
              "consts16": consts16, "vconst": vconst}
    in_maps = []
    for i in range(B):
        m = dict(shared)
        m["xT"] = np.ascontiguousarray(tensor[i].T.astype(np.float16))
        in_maps.append(m)
    return in_maps


def run(tensor, w_qkv, w_proj, trace=False):
    in_maps = make_in_maps(tensor, w_qkv, w_proj)
    nc = _get_nc()
    res = bass_utils.run_bass_kernel_spmd(nc, in_maps, core_ids=list(range(B)),
                                          trace=trace)
    out = np.stack([res.results[i]["out"] for i in range(B)])
    return out, res


def kernel(tensor, w_qkv, w_proj):
    out, _ = run(tensor, w_qkv, w_proj, trace=False)
    return out.astype(np.float32)
